# revision 27
# baseline (speedup 1.0000x reference)
"""JumpAttention Trainium2 kernel (bf16 main path, fp8 agg garnish).

Problem: B=16, S=1024, H=2048, D=256.
  Q/K/K2/V = hs @ W*, 3 biased attentions + 2 aggregation attentions,
  triadic-weighted combine, output projection by Wo.

Strategy:
  - Data-parallel over batch: 2 batches per core on 8 cores; weights and
    pos_bias replicated.
  - bf16 matmuls with fp32 PSUM accumulation on the main path (fp8 there
    fails the accuracy gate: diffuse attention averages the signal down
    as fast as the noise, so per-element fp8 noise stays 2-4% of the
    output).  The aggregation branches are nearly flat softmaxes and
    tolerate fp8: their P tiles and K/K2 value tables run fp8-e4m3 with
    DoubleRow perf mode (half the PE instructions and cycles there).
  - Scores are computed transposed (s^T[k, q]) so that exp(s^T) directly
    serves as the stationary operand of the P@V matmul - no P transpose.
  - Softmax has no max-subtraction; exp(s - 3) keeps the fp8 agg P in
    range (the shift cancels in softmax); the denominator comes from a
    ones-column appended to the value table (row-sums appear as one
    extra output column of the P@V matmul).
  - pos_bias folded multiplicatively: exp(s + b) = exp(s) * exp(b), with
    exp(pos_bias^T) precomputed on host in bf16 (the multiply runs on
    DVE in its 2x all-SBUF bf16 mode).
  - softmax(triadic_weight) computed on host, baked in as immediates.
  - Two-batch software pipeline: emission is interleaved in two lanes so
    batch 1's projection work (PE/DVE, no Act) fills the engine gaps
    under batch 0's attention (Act-heavy), and each batch's final
    projection is fused into its attention units (no drain tail).
  - Constants (weights, exp-bias table, identity) load outside the
    timing loop; build_program(reps=N) wraps the per-iteration body in
    an on-device For_i loop so test.py can measure per-iteration HW time
    with host dispatch overhead amortized away.
"""

import os
from contextlib import ExitStack

import numpy as np
import ml_dtypes

B, S, H, D = 16, 1024, 2048, 256
NCORES = 8
BPC = B // NCORES  # batches per core
P = 128
HT = H // P   # 16 h-tiles
KT = S // P   # 8 s-tiles
DT = D // P   # 2 d-tiles
NQ = 512      # moving free dim (q) chunk
QC = S // NQ  # 2 q chunks
HC = H // NQ  # 4 h chunks in final projection
XG = 8        # h-tiles per x load group

EXPB = -3.0   # exp argument shift (cancels in softmax; keeps fp8 P in range)

LAST_RESULTS = None


def _consts(nc, tc, ctx, mybir, make_identity, handles, w_tri):
    """Pools + loop-invariant constants (emitted outside the reps loop)."""
    xT_h, wq_h, wk_h, wk2_h, wv_h, wo_h, ebT_h, out_h = handles
    dt = mybir.dt
    bf16 = dt.bfloat16
    f32 = dt.float32

    pools = dict(
        consts=ctx.enter_context(tc.tile_pool(name="consts", bufs=1)),
        xpool=ctx.enter_context(tc.tile_pool(name="xpool", bufs=2)),
        actp=ctx.enter_context(tc.tile_pool(name="actp", bufs=1)),
        ppool=ctx.enter_context(tc.tile_pool(name="ppool", bufs=1)),
        tmpp=ctx.enter_context(tc.tile_pool(name="tmpp", bufs=4)),
        psA=ctx.enter_context(tc.tile_pool(name="psA", bufs=4, space="PSUM")),
        psO=ctx.enter_context(tc.tile_pool(name="psO", bufs=3, space="PSUM")),
        psT=ctx.enter_context(tc.tile_pool(name="psT", bufs=1, space="PSUM")),
    )
    consts = pools["consts"]

    wq_sb = consts.tile([P, HT, D], bf16, name="wq_sb")
    wk_sb = consts.tile([P, HT, D], bf16, name="wk_sb")
    wk2_sb = consts.tile([P, HT, D], bf16, name="wk2_sb")
    wv_sb = consts.tile([P, HT, D], bf16, name="wv_sb")
    for t_, h_ in ((wq_sb, wq_h), (wk_sb, wk_h), (wk2_sb, wk2_h), (wv_sb, wv_h)):
        nc.sync.dma_start(out=t_, in_=h_[:].rearrange("(t p) d -> p t d", p=P))
    wo_sb = consts.tile([P, DT, H], bf16, name="wo_sb")
    nc.sync.dma_start(out=wo_sb, in_=wo_h[:].rearrange("(t p) h -> p t h", p=P))
    ebT_sb = consts.tile([P, KT, S], bf16, name="ebT_sb")
    nc.sync.dma_start(out=ebT_sb, in_=ebT_h[:].rearrange("(t p) q -> p t q", p=P))
    identb = consts.tile([P, P], bf16, name="identb")
    make_identity(nc, identb)
    ebias = consts.tile([P, 1], f32, name="ebias")
    nc.gpsimd.memset(ebias, EXPB)
    wconst = consts.tile([P, 4], f32, name="wconst")
    for _i, _w in enumerate((float(w_tri[0]), float(w_tri[1]),
                             float(w_tri[2]), 0.0)):
        nc.gpsimd.memset(wconst[:, _i:_i + 1], _w)
    cst = dict(wq_sb=wq_sb, wk_sb=wk_sb, wk2_sb=wk2_sb, wv_sb=wv_sb,
               wo_sb=wo_sb, ebT_sb=ebT_sb, identb=identb, ebias=ebias,
               wconst=wconst)
    return pools, cst



def _x_loads(nc, xpool, mybir, xT_h, b):
    xgs = []
    for g in range(HT // XG):
        xg = xpool.tile([P, XG, S], mybir.dt.bfloat16, name="xg", tag="xg")
        nc.sync.dma_start(
            out=xg,
            in_=xT_h[b, g * XG * P:(g + 1) * XG * P, :].rearrange(
                "(t p) q -> p t q", p=P
            ),
        )
        xgs.append(xg)
    return xgs


def _body(nc, tc, mybir, handles, pools, cst):
    """One full iteration of the kernel (both batches)."""
    xT_h, wq_h, wk_h, wk2_h, wv_h, wo_h, ebT_h, out_h = handles
    dt = mybir.dt
    bf16 = dt.bfloat16
    f32 = dt.float32
    fp8 = dt.float8e4
    DR = mybir.MatmulPerfMode.DoubleRow
    Exp = mybir.ActivationFunctionType.Exp
    SSC = float(D) ** -0.5

    xpool, actp, ppool, tmpp = (pools[k] for k in
                                ("xpool", "actp", "ppool", "tmpp"))
    psA, psO, psT = (pools[k] for k in ("psA", "psO", "psT"))
    wq_sb, wk_sb, wk2_sb, wv_sb, wo_sb, ebT_sb, identb, ebias, wconst = (
        cst[k] for k in ("wq_sb", "wk_sb", "wk2_sb", "wv_sb", "wo_sb",
                         "ebT_sb", "identb", "ebias", "wconst"))

    def gen_A(b, xgs, T):
        """x projections, V, K'/K2' fp8 row tables. No Act-engine work."""
        xts = [xgs[h_t // XG][:, h_t % XG, :] for h_t in range(HT)]

        kT = actp.tile([P, DT, S], bf16, name="kT", tag="kT", bufs=2)
        k2T = actp.tile([P, DT, S], bf16, name="k2T", tag="k2T", bufs=2)
        qT = actp.tile([P, DT, S], bf16, name="qT", tag="qT", bufs=2)
        T.update(qT=qT, kT=kT, k2T=k2T)

        def proj_cols(dst, w_sb):
            for d_t in range(DT):
                for q_c in range(QC):
                    ps = psA.tile([P, NQ], f32, name="ps_proj", tag="psA")
                    for h_t in range(HT):
                        nc.tensor.matmul(
                            ps,
                            lhsT=w_sb[:, h_t, d_t * P:(d_t + 1) * P],
                            rhs=xts[h_t][:, q_c * NQ:(q_c + 1) * NQ],
                            start=(h_t == 0),
                            stop=(h_t == HT - 1),
                        )
                    nc.vector.tensor_copy(dst[:, d_t, q_c * NQ:(q_c + 1) * NQ], ps)

        kT8 = actp.tile([P, DT, S], fp8, name="kT8", tag="kT8", bufs=2)
        k2T8 = actp.tile([P, DT, S], fp8, name="k2T8", tag="k2T8", bufs=2)
        T.update(kT8=kT8, k2T8=k2T8)
        proj_cols(kT, wk_sb)
        nc.vector.tensor_copy(kT8, kT)
        yield
        proj_cols(k2T, wk2_sb)
        nc.vector.tensor_copy(k2T8, k2T)
        yield

        # K', K2' row-layout fp8 value tables (+ ones col) for the agg PV;
        # bf16 transposes of K^T/K2^T, fp8 conversion in the copy.
        kP = actp.tile([P, KT, D + 1], fp8, name="kP", tag="kP", bufs=2)
        k2P = actp.tile([P, KT, D + 1], fp8, name="k2P", tag="k2P", bufs=2)
        T.update(kP=kP, k2P=k2P)
        for src, dst in ((kT, kP), (k2T, k2P)):
            for s_t in range(KT):
                pt = psT.tile([P, DT, P], bf16, name="pt_k", tag="psT")
                for d_t in range(DT):
                    nc.tensor.transpose(
                        pt[:, d_t, :], src[:, d_t, s_t * P:(s_t + 1) * P], identb
                    )
                nc.vector.tensor_copy(dst[:, s_t, :D], pt)
            nc.gpsimd.memset(dst[:, :, D:D + 1], 1.0)
        yield

        proj_cols(qT, wq_sb)
        yield

        # V' : [s-part, d + ones-column] bf16 (x as stationary)
        vP = actp.tile([P, KT, D + 1], bf16, name="vP", tag="vP", bufs=2)
        T.update(vP=vP)
        for s_t in range(KT):
            ps = psA.tile([P, NQ], f32, name="ps_v", tag="psA")
            for h_t in range(HT):
                nc.tensor.matmul(
                    ps[:, :D],
                    lhsT=xts[h_t][:, s_t * P:(s_t + 1) * P],
                    rhs=wv_sb[:, h_t, :],
                    start=(h_t == 0),
                    stop=(h_t == HT - 1),
                )
            nc.vector.tensor_copy(vP[:, s_t, :D], ps[:, :D])
            if s_t == KT // 2 - 1:
                yield
        nc.gpsimd.memset(vP[:, :, D:D + 1], 1.0)
        yield

    def gen_B(b, T):
        """agg attentions + 3 biased attentions + combine + final proj."""

        def agg_branch(colT_k, rowT_k, colP_k, outT):
            # scores bf16; P fp8; PV fp8 DoubleRow against the fp8 table
            for q_c in range(QC):
                colT, rowT, colP = T[colT_k], T[rowT_k], T[colP_k]
                pch = ppool.tile([P, KT, NQ], fp8, name="pch_a", tag="pA",
                                 bufs=2)
                for m_t in range(KT):
                    ps = psA.tile([P, NQ], f32, name="ps_as", tag="psA")
                    nc.tensor.matmul(
                        ps,
                        lhsT=colT[:, 0:DT, m_t * P:(m_t + 1) * P],
                        rhs=rowT[:, 0:DT, q_c * NQ:(q_c + 1) * NQ],
                        start=True,
                        stop=True,
                        perf_mode=DR,
                    )
                    nc.scalar.activation(pch[:, m_t, :], ps, Exp, scale=SSC,
                                         bias=ebias)
                for q_t in range(NQ // P):
                    po = psO.tile([P, D + 1], f32, name="po_a", tag="psO")
                    for jm in range(KT // 2):
                        nc.tensor.matmul(
                            po,
                            lhsT=pch[:, 2 * jm:2 * jm + 2, q_t * P:(q_t + 1) * P],
                            rhs=colP[:, 2 * jm:2 * jm + 2, :],
                            start=(jm == 0),
                            stop=(jm == KT // 2 - 1),
                            perf_mode=DR,
                        )
                    rec = tmpp.tile([P, 1], f32, name="rec_a", tag="rec_a")
                    nc.vector.reciprocal(rec, po[:, D:D + 1])
                    sd = tmpp.tile([P, D], bf16, name="sd_a", tag="sd_a")
                    nc.vector.tensor_scalar_mul(sd, po[:, :D], rec)
                    s_t = q_c * (NQ // P) + q_t
                    pt = psT.tile([P, DT, P], bf16, name="pt_a", tag="psT")
                    for d_t in range(DT):
                        nc.tensor.transpose(
                            pt[:, d_t, :], sd[:, d_t * P:(d_t + 1) * P], identb
                        )
                    nc.vector.tensor_copy(
                        outT[:, 0:DT, s_t * P:(s_t + 1) * P], pt
                    )
                yield

        kaT = actp.tile([P, DT, S], bf16, name="kaT", tag="kaT")
        T.update(kaT=kaT)
        yield from agg_branch("k2T8", "kT8", "k2P", kaT)   # K att K2 -> K_agg
        k2aT = actp.tile([P, DT, S], bf16, name="k2aT", tag="k2aT")
        T.update(k2aT=k2aT)
        yield from agg_branch("kT8", "k2T8", "kP", k2aT)   # K2 att K -> K2_agg

        combT = actp.tile([P, DT, S], bf16, name="combT", tag="combT")
        T.update(combT=combT)
        for q_c in range(QC):
            qT, vP = T["qT"], T["vP"]
            kTs = (T["kT"], kaT, k2aT)
            pchs = []
            for i in range(3):
                pch = ppool.tile([P, KT, NQ], bf16, name="pch_b", tag="pB",
                                 bufs=3)
                for m_t in range(KT):
                    ps = psA.tile([P, NQ], f32, name="ps_bs", tag="psA")
                    for d_t in range(DT):
                        nc.tensor.matmul(
                            ps,
                            lhsT=kTs[i][:, d_t, m_t * P:(m_t + 1) * P],
                            rhs=qT[:, d_t, q_c * NQ:(q_c + 1) * NQ],
                            start=(d_t == 0),
                            stop=(d_t == DT - 1),
                        )
                    et = tmpp.tile([P, NQ], bf16, name="et", tag="et", bufs=3)
                    nc.scalar.activation(et, ps, Exp, scale=SSC, bias=ebias)
                    nc.vector.tensor_mul(
                        pch[:, m_t, :], et,
                        ebT_sb[:, m_t, q_c * NQ:(q_c + 1) * NQ],
                    )
                pchs.append(pch)
                yield
            for q_t in range(NQ // P):
                pos = []
                for i in range(3):
                    po = psO.tile([P, D + 1], f32, name="po_b", tag="psO")
                    for m_t in range(KT):
                        nc.tensor.matmul(
                            po,
                            lhsT=pchs[i][:, m_t, q_t * P:(q_t + 1) * P],
                            rhs=vP[:, m_t, :],
                            start=(m_t == 0),
                            stop=(m_t == KT - 1),
                        )
                    pos.append(po)
                rec = tmpp.tile([P, 4], f32, name="rec_b", tag="rec_b")
                for i in range(3):
                    nc.vector.reciprocal(rec[:, i:i + 1], pos[i][:, D:D + 1])
                recw = tmpp.tile([P, 4], f32, name="recw", tag="recw")
                nc.vector.tensor_mul(recw, rec, wconst)
                accs = []
                for i in range(3):
                    acc = tmpp.tile([P, D], f32, name="acc", tag=f"acc{i}",
                                    bufs=2)
                    nc.scalar.mul(acc, pos[i][:, :D], recw[:, i:i + 1])
                    accs.append(acc)
                t01 = tmpp.tile([P, D], f32, name="t01", tag="t01")
                nc.gpsimd.tensor_add(t01, accs[0], accs[1])
                comb = tmpp.tile([P, D], bf16, name="comb", tag="comb")
                nc.gpsimd.tensor_add(comb, t01, accs[2])
                s_t = q_c * (NQ // P) + q_t
                pt = psT.tile([P, DT, P], bf16, name="pt_c", tag="psT")
                for d_t in range(DT):
                    nc.tensor.transpose(
                        pt[:, d_t, :], comb[:, d_t * P:(d_t + 1) * P], identb
                    )
                nc.vector.tensor_copy(
                    combT[:, 0:DT, s_t * P:(s_t + 1) * P], pt
                )
            yield
            # fused final projection for this q_c's four s_t rows
            for s_t in range(q_c * (NQ // P), (q_c + 1) * (NQ // P)):
                ostage = tmpp.tile([P, H], bf16, name="ostage", tag="ostage",
                                   bufs=2)
                for h_c in range(HC):
                    ps = psA.tile([P, NQ], f32, name="ps_o", tag="psA")
                    for d_t in range(DT):
                        nc.tensor.matmul(
                            ps,
                            lhsT=combT[:, d_t, s_t * P:(s_t + 1) * P],
                            rhs=wo_sb[:, d_t, h_c * NQ:(h_c + 1) * NQ],
                            start=(d_t == 0),
                            stop=(d_t == DT - 1),
                        )
                    if h_c % 2 == 0:
                        nc.scalar.copy(ostage[:, h_c * NQ:(h_c + 1) * NQ], ps)
                    else:
                        nc.vector.tensor_copy(
                            ostage[:, h_c * NQ:(h_c + 1) * NQ], ps
                        )
                nc.sync.dma_start(
                    out=out_h[b, s_t * P:(s_t + 1) * P, :], in_=ostage
                )
            yield

    def noops(n):
        for _ in range(n):
            yield

    def interleave(*gens):
        live = list(gens)
        while live:
            for g in list(live):
                try:
                    next(g)
                except StopIteration:
                    live.remove(g)

    def xload_unit(b, T):
        xgs = []
        for g in range(HT // XG):
            xg = xpool.tile([P, XG, S], bf16, name="xg", tag="xg")
            nc.sync.dma_start(
                out=xg,
                in_=xT_h[b, g * XG * P:(g + 1) * XG * P, :].rearrange(
                    "(t p) q -> p t q", p=P
                ),
            )
            xgs.append(xg)
        T["xgs"] = xgs
        yield

    # two emission lanes: lane1 = projection-type work (PE/DVE, no Act),
    # lane2 = attention work (Act-heavy).  Alternating units keep every
    # engine's queue supplied; noops delay lane2 until its batch-0 inputs
    # (kT/k2T/kP/k2P) are emitted.
    T0, T1 = {}, {}
    xgs0 = _x_loads(nc, xpool, mybir, xT_h, 0)

    def lane1():
        yield from gen_A(0, xgs0, T0)
        yield from xload_unit(1, T1)
        yield from gen_A(1, T1["xgs"], T1)

    def lane2():
        yield from noops(3)
        yield from gen_B(0, T0)
        yield from gen_B(1, T1)

    interleave(lane1(), lane2())


def build_program(w_tri, reps=1):
    """reps>1 wraps the kernel body in an on-device For_i loop executing the
    identical per-iteration work back-to-back; used by test.py to measure
    per-iteration HW time with host dispatch overhead amortized away."""
    import concourse.bacc as bacc
    import concourse.tile as tile
    from concourse import mybir
    from concourse.masks import make_identity

    nc = bacc.Bacc()
    dt = mybir.dt
    bf16 = dt.bfloat16
    xT_h = nc.dram_tensor("xT", [BPC, H, S], bf16, kind="ExternalInput")
    wq_h = nc.dram_tensor("wq", [H, D], bf16, kind="ExternalInput")
    wk_h = nc.dram_tensor("wk", [H, D], bf16, kind="ExternalInput")
    wk2_h = nc.dram_tensor("wk2", [H, D], bf16, kind="ExternalInput")
    wv_h = nc.dram_tensor("wv", [H, D], bf16, kind="ExternalInput")
    wo_h = nc.dram_tensor("wo", [D, H], bf16, kind="ExternalInput")
    ebT_h = nc.dram_tensor("ebT", [S, S], bf16, kind="ExternalInput")
    out_h = nc.dram_tensor("out", [BPC, S, H], bf16, kind="ExternalOutput")
    handles = (xT_h, wq_h, wk_h, wk2_h, wv_h, wo_h, ebT_h, out_h)

    with ExitStack() as ctx:
        tc = ctx.enter_context(tile.TileContext(nc))
        pools, cst = _consts(nc, tc, ctx, mybir, make_identity, handles, w_tri)
        if reps == 1:
            _body(nc, tc, mybir, handles, pools, cst)
        else:
            with tc.For_i(0, reps):
                _body(nc, tc, mybir, handles, pools, cst)
    nc.compile()
    return nc


def prep_inputs(hidden_states, Wq, Wk, Wk2, Wv, Wo, triadic_weight, pos_bias):
    f32 = np.float32
    bf16 = ml_dtypes.bfloat16

    t = np.asarray(triadic_weight, dtype=np.float64)
    e = np.exp(t - t.max())
    w_tri = (e / e.sum()).astype(f32)

    wq_np = np.asarray(Wq, f32).astype(bf16)
    wk_np = np.asarray(Wk, f32).astype(bf16)
    wk2_np = np.asarray(Wk2, f32).astype(bf16)
    wv_np = np.asarray(Wv, f32).astype(bf16)
    wo_np = np.asarray(Wo, f32).astype(bf16)
    ebT_np = np.exp(np.asarray(pos_bias, f32).T).astype(bf16)
    hs = np.asarray(hidden_states, f32)
    xTs = [
        np.ascontiguousarray(hs[c * BPC:(c + 1) * BPC].transpose(0, 2, 1)).astype(bf16)
        for c in range(NCORES)
    ]
    in_maps = [
        {
            "xT": xTs[c],
            "wq": wq_np,
            "wk": wk_np,
            "wk2": wk2_np,
            "wv": wv_np,
            "wo": wo_np,
            "ebT": ebT_np,
        }
        for c in range(NCORES)
    ]
    return w_tri, in_maps


def kernel(hidden_states, Wq, Wk, Wk2, Wv, Wo, triadic_weight, pos_bias):
    global LAST_RESULTS
    from concourse.bass_utils import run_bass_kernel_spmd

    f32 = np.float32
    w_tri, in_maps = prep_inputs(
        hidden_states, Wq, Wk, Wk2, Wv, Wo, triadic_weight, pos_bias
    )
    nc = build_program(w_tri)

    if os.environ.get("KERNEL_BUILD_ONLY"):
        return np.zeros((B, S, H), f32)

    res = run_bass_kernel_spmd(nc, in_maps, core_ids=list(range(NCORES)))
    LAST_RESULTS = res
    if res.exec_time_ns:
        print(f"HW exec time: {res.exec_time_ns} ns")
    out = np.concatenate([r["out"] for r in res.results], axis=0)
    return np.ascontiguousarray(out.astype(f32))
